# revision 5
# baseline (speedup 1.0000x reference)
"""Trainium2 TT-structured kernel for nn_KerasDense_32263794328408.

y = relu(x @ M + b), M = TT-matrix, ranks [1,8,8,8,1], modes 8x8x8x8.

Algebra: merging cores (0,1) -> A[i12, j12, r] (64x64x8) and cores
(2,3) -> B[r, i34, j34] (8x64x64) gives M = sum_r A_r (x) B_r (a
rank-8 Kronecker sum). Contracting B first then A needs 34.4 GFLOP
total vs 137.4 GFLOP dense -- 4x fewer. ~125 us vs the 237 us dense
near-roofline GEMM this replaced (1.9x).

Two-stage PE schedule per core (batch-sharded 8 ways, B_L=512):

Stage 1 (contract i34, K=64): z[b, i12, r, j34] = sum_i34 x[b,i12,i34]
  * B[r,i34,j34]. Stationary = x'' chunk [64 i34, 128 (b-hat,i12)]
  (one chunk = 2 batch rows x 64 i12), moving = Bmat [64, 512 (r,j34)].
  K=64 would waste half the array, so two chunks run CONCURRENTLY via
  tile_position row-tiling (rows 0-63 / 64-127, separate PSUM banks;
  measured 3ns apart). 256 chunks -> 128 windows at full PE rate.

Stage 2 (contract (i12, r), K=512): y[b, j12, j34] = sum A z. r rides
  the PSUM accumulation loop (8 matmuls per bank), i12 rides the
  partitions: stationary At_r = diag(A_r, A_r) block-diagonal so the
  two batch rows of a chunk don't mix. Each r-matmul streams all 8
  pairs of a bank in one N=512 pass via a 3D strided rhs [128, s=8,
  64] into the shared z-bank tile. Bias = one K=64 matmul (duplicated
  identity stationary x bias rows) opening each bank's group; its
  [64,128] stationary shape matches the windows so it rides in their
  LDWEIGHTS stream.

z (16.8M elem/core) must cross PSUM->SBUF through compute engines
(DMA has no PSUM port, GPSIMD no PSUM access, PSUM reads force DVE
1x mode): one fused [128,1024] copy per window, alternating ACT/DVE,
with the relu+bf16 cast of finished y banks 2:1 on ACT/DVE. Both
engines run ~83% busy under the PE.

Schedule: units of [winA winB bias][r0-r3][winC winD][r4-r7+relu],
step2 lagging its z-bank by 2 units. This grouping holds the PE
shape-transition count at 4/unit (each win<->step2 switch exposes
~100ns of LDWEIGHTS); 3-window bursts overrun the 3-deep [128,1024]
PSUM pool (drains pace at ~1.2us/window). Steady state: 3227 ns/unit,
PE fully dense. Warmup: 40 HAM-lift matmuls on a memset tile (no DMA
dependency) bridge the ~7-11us preamble so the clock gate opens
before the first real matmul; first xpp segment is split 4-ways so
window 0's data lands early. Units 0-1 (drain-paced, no step2 yet)
get warm-filler matmul bursts between window pairs so the HAM MID
window never re-throttles the PE to 1.2 GHz as the steady stream
begins (~123.6-125.3 us measured).

y is stored bank-contiguous ([128, 512] blocks, fully-contiguous
128 KB store DMAs on the sync ring) and un-permuted on the host.

Everything ships bf16; accumulation is fp32 in PSUM. absmax/scale =
4.4e-3 (gate 2e-2); fp8 anywhere pushes past the gate (z or step2 in
e4m3 -> ~5% relative error over the K=512 contraction).
"""

import sys

if "/opt/trn_rl_repo" not in sys.path:
    sys.path.insert(0, "/opt/trn_rl_repo")

import ml_dtypes
import numpy as np

import concourse.bacc as bacc
import concourse.bass as bass
import concourse.mybir as mybir
import concourse.tile as tile
from concourse.bass_utils import run_bass_kernel_spmd

F32 = mybir.dt.float32
BF16 = mybir.dt.bfloat16
NP_BF16 = ml_dtypes.bfloat16

B_FULL = 4096
F_FULL = 4096
O_FULL = 4096
N_CORES = 8
B_L = B_FULL // N_CORES          # 512 batch rows per core

NPAIR = B_L // 2                 # 256 chunks (= batch pairs) per core
NBCH = 8                         # bchunks
PAIR_PER_BCH = NPAIR // NBCH     # 32
NWIN = PAIR_PER_BCH // 2         # 16 step1 windows per bchunk
NBANK = PAIR_PER_BCH // 8        # 4 step2 banks per bchunk
R = 8

_CACHE: dict = {}


def _build_module() -> bass.Bass:
    nc = bacc.Bacc(None, target_bir_lowering=False)

    # x'': row-tiled stationary chunks. Column block w = 128 cols holds
    # chunks 2w (partitions 0-63) and 2w+1 (partitions 64-127); within a
    # block, col m = bhat*64 + i12, partition (c%2)*64 + i34.
    xpp = nc.declare_dram_parameter("xpp", [128, NPAIR * 64], BF16, isOutput=False)
    # Bmat moving operand, both partition halves identical:
    # bmat[h*64 + i34, r*64 + j34] = B[r, i34, j34]
    bmat = nc.declare_dram_parameter("bmat", [128, 512], BF16, isOutput=False)
    # At_r block-diagonal stationaries, concat along free dim.
    atil = nc.declare_dram_parameter("atil", [128, R * 128], BF16, isOutput=False)
    # Bias: one K=64 matmul per bank. eye[j12p, bhat*64+j12]=delta,
    # brhs[j12p, s*64+j34] = bias[j12p*64+j34].
    eye = nc.declare_dram_parameter("eye", [128, 128], BF16, isOutput=False)
    brhs = nc.declare_dram_parameter("brhs", [128, 512], BF16, isOutput=False)
    # y stored bank-major: row g*128 + (bhat*64 + j12), col s*64 + j34;
    # host un-permutes. Keeps every store DMA fully contiguous.
    y = nc.declare_dram_parameter("y", [32 * 128, 512], BF16, isOutput=True)

    with tile.TileContext(nc) as tc:
        with (
            tc.tile_pool(name="xt", bufs=1) as x_pool,
            tc.tile_pool(name="cst", bufs=1) as c_pool,
            tc.tile_pool(name="z1", bufs=1) as z_pool,
            tc.tile_pool(name="ysb", bufs=1) as y_pool,
            tc.tile_pool(name="ps1", bufs=3, space="PSUM") as ps1_pool,
            tc.tile_pool(name="ps2", bufs=2, space="PSUM") as ps2_pool,
        ):
            # Warmup operand via memset: no DMA dependency, so the HAM
            # warmup matmuls start right after engine init (~4us) instead
            # of waiting for the first DMA (~10us).
            ones_sb = c_pool.tile([128, 128], BF16, tag="ones")
            nc.gpsimd.memset(ones_sb[:], 1.0)
            bmat_sb = c_pool.tile([128, 512], BF16, tag="bmat")
            nc.sync.dma_start(out=bmat_sb[:], in_=bmat[:])

            # x'' in 8 bchunk pieces; first two segments ahead of the
            # step2 constants (not needed until unit 2).
            xpp_sb = x_pool.tile([128, NPAIR * 64], BF16, tag="xpp")
            seg = PAIR_PER_BCH * 64
            sub = seg // 4
            for j in range(4):
                nc.sync.dma_start(
                    out=xpp_sb[:, j * sub : (j + 1) * sub],
                    in_=xpp[:, j * sub : (j + 1) * sub],
                )
            nc.sync.dma_start(
                out=xpp_sb[:, seg : 2 * seg], in_=xpp[:, seg : 2 * seg]
            )
            eye_sb = c_pool.tile([128, 128], BF16, tag="eye")
            nc.sync.dma_start(out=eye_sb[:], in_=eye[:])
            brhs_sb = c_pool.tile([128, 512], BF16, tag="brhs")
            nc.sync.dma_start(out=brhs_sb[:], in_=brhs[:])
            atil_sb = c_pool.tile([128, R * 128], BF16, tag="atil")
            nc.sync.dma_start(out=atil_sb[:], in_=atil[:])
            for k in range(2, NBCH):
                nc.sync.dma_start(
                    out=xpp_sb[:, k * seg : (k + 1) * seg],
                    in_=xpp[:, k * seg : (k + 1) * seg],
                )

            # HAM warmup: full-K dummy matmuls while the first loads land.
            warm = ps2_pool.tile([128, 512], F32, tag="ps2", name="warm")
            for i in range(40):
                nc.tensor.matmul(
                    warm[:, 0:128],
                    ones_sb[:],
                    ones_sb[:],
                    start=(i == 0),
                    stop=(i == 39),
                )

            ZB = 4  # z_bank tiles in flight (write g, drain g/g-1, read g-2)
            z_banks = [None] * ZB

            def step1_window(w):
                # window w: chunks 2w (rows 0-63), 2w+1 (rows 64-127),
                # both -> one [128, 1024] psum pair, one fused drain into
                # z_bank[w//4] cols (2w%8)*512 .. +1024.
                g = w // 4
                if w % 4 == 0:
                    zb = z_pool.tile(
                        [128, 8 * 512], BF16, tag=f"zb{g % ZB}",
                        name=f"zb_{g}"
                    )
                    z_banks[g % ZB] = zb
                zb = z_banks[g % ZB]
                ps = ps1_pool.tile([128, 1024], F32, tag="ps1",
                                   name=f"z_{w}")
                for half in (0, 1):
                    nc.tensor.matmul(
                        ps[:, half * 512 : half * 512 + 512],
                        xpp_sb[half * 64 : half * 64 + 64,
                               w * 128 : (w + 1) * 128],
                        bmat_sb[half * 64 : half * 64 + 64, :],
                        start=True,
                        stop=True,
                        tile_position=(half * 64, 0),
                    )
                dst = zb[:, (2 * w % 8) * 512 : (2 * w % 8) * 512 + 1024]
                if w % 2 == 0:
                    nc.scalar.copy(dst, ps[:])
                else:
                    nc.vector.tensor_copy(dst, ps[:])

            y_ps = {}

            def step2_piece(g, piece):
                # bank g's PE work, split into 5 pieces interleaved between
                # step1 windows: [bias], [r0 r1], [r2 r3], [r4 r5], [r6 r7
                # + relu + store].
                zb = z_banks[g % ZB]
                if piece == 0:
                    ps = ps2_pool.tile([128, 512], F32, tag="ps2",
                                       name=f"y_{g}")
                    y_ps[g % 2] = ps
                    nc.tensor.matmul(
                        ps[:], eye_sb[0:64, :], brhs_sb[0:64, :],
                        start=True, stop=False,
                        skip_group_check=True,
                    )
                    return
                ps = y_ps[g % 2]
                z3 = zb[:].rearrange("p (s q) -> p s q", q=512)
                for r in (2 * piece - 2, 2 * piece - 1):
                    nc.tensor.matmul(
                        ps[:],
                        atil_sb[:, r * 128 : (r + 1) * 128],
                        z3[:, :, r * 64 : (r + 1) * 64],
                        start=False,
                        stop=(r == R - 1),
                        skip_group_check=True,
                    )
                if piece == 4:
                    ysb = y_pool.tile(
                        [128, 512], BF16, tag=f"y{g % 4}", name=f"ysb_{g}"
                    )
                    if g % 3 != 0:
                        nc.scalar.activation(
                            ysb[:], ps[:], mybir.ActivationFunctionType.Relu
                        )
                    else:
                        nc.vector.tensor_scalar_max(ysb[:], ps[:], 0.0)
                    nc.sync.dma_start(
                        out=y[g * 128 : (g + 1) * 128, :], in_=ysb[:]
                    )

            NG = NPAIR // 8  # 32 banks
            # Unit u: windows 4u..4u+3 + step2 of bank u-2, grouped to
            # minimize PE shape transitions (each win<->step2 switch costs
            # ~100ns of exposed LDWEIGHTS): [winA winB bias][r0-r3]
            # [winC winD][r4-r7 relu].
            def warm_fill(n):
                # Ramp filler: units 0-1 are drain-paced and the PE idles
                # between windows; without these the HAM MID window can
                # re-throttle the clock to 1.2 GHz right as the steady
                # stream begins.
                for i in range(n):
                    nc.tensor.matmul(
                        warm[:, 0:128], ones_sb[:], ones_sb[:],
                        start=(i == 0), stop=(i == n - 1),
                    )

            for u in range(NG + 2):
                if u < NG:
                    step1_window(4 * u)
                    step1_window(4 * u + 1)
                if u < 2:
                    warm_fill(8)
                if u >= 2:
                    step2_piece(u - 2, 0)   # bias
                    step2_piece(u - 2, 1)   # r0 r1
                    step2_piece(u - 2, 2)   # r2 r3
                if u < NG:
                    step1_window(4 * u + 2)
                    step1_window(4 * u + 3)
                if u < 2:
                    warm_fill(8)
                if u >= 2:
                    step2_piece(u - 2, 3)   # r4 r5
                    step2_piece(u - 2, 4)   # r6 r7 + relu + store

    nc.finalize()
    return nc


def _merge_cores(core0, core1, core2, core3):
    g1 = core0[0].astype(np.float64)            # [m1, n1, r1]
    t12 = np.einsum("mnr,rMNs->mMnNs", g1, core1.astype(np.float64))
    A = t12.reshape(64, 64, 8)                  # [i12, j12, r]
    g4 = core3[..., 0].astype(np.float64)       # [r3, m4, n4]
    t34 = np.einsum("rmns,sMN->rmMnN", core2.astype(np.float64), g4)
    B = t34.reshape(8, 64, 64)                  # [r, i34, j34]
    return A.astype(np.float32), B.astype(np.float32)


def _prepare_in_maps(x, A, B, bias):
    # Shared constants
    bm = np.ascontiguousarray(
        np.tile(B.transpose(1, 0, 2).reshape(64, 512), (2, 1))
    ).astype(NP_BF16)                            # [128, 512]
    at = np.zeros((R, 128, 128), dtype=np.float32)
    for r in range(R):
        at[r, 0:64, 0:64] = A[:, :, r]
        at[r, 64:128, 64:128] = A[:, :, r]
    at = np.ascontiguousarray(
        at.transpose(1, 0, 2).reshape(128, R * 128)
    ).astype(NP_BF16)
    eyev = np.ascontiguousarray(
        np.tile(np.eye(64, dtype=np.float32), (2, 2))
    ).astype(NP_BF16)                            # [128, 128]
    brhs = np.ascontiguousarray(
        np.tile(bias.reshape(64, 64), (2, 8))
    ).astype(NP_BF16)                            # [128, 512]

    in_maps = []
    xr = x.reshape(B_FULL, 64, 64)               # [b, i12, i34]
    for c in range(N_CORES):
        xc = xr[c * B_L : (c + 1) * B_L]         # [512, 64, 64]
        # t[pair, i34, bhat*64+i12]
        t = xc.reshape(NPAIR, 2, 64, 64).transpose(0, 3, 1, 2).reshape(
            NPAIR, 64, 128
        )
        # xpp[(codd*64 + i34), blk*128 + m] = t[blk*2 + codd, i34, m]
        t2 = t.reshape(NPAIR // 2, 2, 64, 128)
        xpp = np.ascontiguousarray(
            t2.transpose(1, 2, 0, 3).reshape(128, NPAIR * 64)
        ).astype(NP_BF16)
        in_maps.append(
            {
                "xpp": xpp,
                "bmat": bm,
                "atil": at,
                "eye": eyev,
                "brhs": brhs,
            }
        )
    return in_maps


def kernel(x, core0, core1, core2, core3, b) -> np.ndarray:
    x = np.asarray(x, dtype=np.float32)
    A, B = _merge_cores(
        np.asarray(core0, dtype=np.float32),
        np.asarray(core1, dtype=np.float32),
        np.asarray(core2, dtype=np.float32),
        np.asarray(core3, dtype=np.float32),
    )
    bias = np.asarray(b, dtype=np.float32)

    if "nc" not in _CACHE:
        _CACHE["nc"] = _build_module()
    nc = _CACHE["nc"]

    in_maps = _prepare_in_maps(x, A, B, bias)
    res = run_bass_kernel_spmd(nc, in_maps, core_ids=list(range(N_CORES)))

    y = np.empty((B_FULL, O_FULL), dtype=np.float32)
    for c in range(N_CORES):
        arr = np.asarray(res.results[c]["y"]).astype(np.float32)
        # [g, bhat, j12, s, j34] -> b_local = 16g + 2s + bhat
        t = arr.reshape(32, 2, 64, 8, 64).transpose(0, 3, 1, 2, 4)
        y[c * B_L : (c + 1) * B_L] = t.reshape(B_L, O_FULL)
    return y
